# revision 23
# baseline (speedup 1.0000x reference)
"""Multi-head self-attention Trainium2 kernel (8-core token-parallel with
collective K/V sharing).

Problem: B=2, N=2048, C=1024, H=16 heads, HD=64.

Sharding: data-parallel over tokens -- core c owns 512 tokens (batch c//4,
block c%4). Each core:

  1. Computes Q, K, V for its OWN 512 tokens only (all 16 heads). Weights
     are Const tensors baked into the NEFF (weight-stationary); the only
     per-call input is the core's 1MB bf16 x^T slice.
  2. AllGathers K (channel-major) and V (natural token-major layout,
     transposed locally on the PE before the gather) across the 4 cores of
     its batch, so every core sees the full 2048-token K/V.
  3. Runs full softmax attention for its 512 queries (no max subtraction:
     |s*scale| < ~10 for this distribution) and the output projection for
     its 512 rows. Outputs are disjoint -- the host just concatenates.

Matmuls run bf16 for QKV, f32r for scores, bf16 for attn@v and the
projection. Per-head softmax denominators come for free as row 64 of the
attn@v accumulator via a constant-ones column appended to v.

`reps` repeats the whole body inside one NEFF so a test harness can
measure device execution time as a slope, cancelling dispatch overhead.
"""

import numpy as np

B, N, C = 2, 2048, 1024
H = 16
HD = C // H  # 64
SCALE = HD ** -0.5
NCORES = 8
QB = 512                      # tokens owned per core
NB = N // QB                  # 4 cores per batch

_CACHE = {}


def _build_program(w_qkv, b_qkv, w_proj, b_proj, reps=1):
    import ml_dtypes
    import concourse.bass as bass
    import concourse.mybir as mybir
    import concourse.tile as tile
    from concourse import bacc

    f32 = mybir.dt.float32
    f32r = mybir.dt.float32r
    bf16 = mybir.dt.bfloat16
    Exp = mybir.ActivationFunctionType.Exp
    Mult = mybir.AluOpType.mult

    nc = bacc.Bacc("TRN2", target_bir_lowering=False, debug=False,
                   num_devices=NCORES)

    # per-call input: this core's own 512 tokens, transposed (bf16)
    xT_d = nc.dram_tensor("xqT", [C, QB], bf16, kind="ExternalInput")
    out_d = nc.dram_tensor("out_q", [QB, C], bf16, kind="ExternalOutput")

    # baked weights (identical on every core)
    wqkv_d = nc.inline_tensor(
        np.ascontiguousarray(w_qkv).astype(ml_dtypes.bfloat16),
        name="wqkv_c")
    w2_d = nc.inline_tensor(
        np.ascontiguousarray(w_proj).astype(ml_dtypes.bfloat16), name="w2_c")
    bqkv_d = nc.inline_tensor(np.ascontiguousarray(
        b_qkv.reshape(24, 128).T, np.float32), name="bqkv_c")
    bproj_d = nc.inline_tensor(
        b_proj.reshape(1, C).astype(ml_dtypes.bfloat16), name="bproj_c")
    id_d = nc.inline_tensor(np.eye(128, dtype=np.float32), name="ident_c")
    ones64_d = nc.inline_tensor(np.ones((1, 64), np.float32), name="o64_c")
    onesr_d = nc.inline_tensor(np.ones((1, 128), ml_dtypes.bfloat16),
                               name="or_c")
    ones16_d = nc.inline_tensor(np.ones((128, 256), ml_dtypes.bfloat16),
                                name="o16_c")

    CC = C // 128      # 8 contraction chunks
    NOC = C // 128     # 8 output chunks per projection (q, k or v)
    NKC = N // 128     # 16 key chunks (full batch)
    NP = H // 2        # 8 head pairs (one 128-row chunk each)

    with tile.TileContext(nc) as tc:
        with tc.tile_pool(name="persist", bufs=1) as persist, \
             tc.tile_pool(name="xt", bufs=1) as xt_pool, \
             tc.tile_pool(name="wst", bufs=2) as wst_pool, \
             tc.tile_pool(name="loc", bufs=2) as loc_pool, \
             tc.tile_pool(name="exp", bufs=3) as exp_pool, \
             tc.tile_pool(name="vld", bufs=2) as vld_pool, \
             tc.tile_pool(name="small", bufs=2) as small_pool, \
             tc.tile_pool(name="ob", bufs=2) as out_pool, \
             tc.tile_pool(name="dram", bufs=1, space="DRAM") as dram, \
             tc.tile_pool(name="ps", bufs=2, space="PSUM") as psum_s, \
             tc.tile_pool(name="po", bufs=2, space="PSUM") as psum_o, \
             tc.tile_pool(name="pr", bufs=2, space="PSUM") as psum_r:

            kT = persist.tile([128, NP, N], f32r, tag="kT")
            qT = persist.tile([128, NP, QB], f32r, tag="qT")
            # natural-layout v per key chunk: per head [v(64) | 1.0]
            v_nat = persist.tile([128, NKC, H * 65], bf16, tag="v_nat")
            w2_sb = persist.tile([128, CC, C], bf16, tag="w2_sb")
            ohT = persist.tile([128, NP, QB], bf16, tag="ohT")
            vsb = persist.tile([128, NOC, QB], f32, tag="vsb")
            b_sb = persist.tile([128, 24], f32, tag="b_sb")
            bproj = persist.tile([1, C], bf16, tag="bproj")
            ident = persist.tile([128, 128], f32, tag="ident")
            ones64 = persist.tile([1, 64], f32r, tag="ones64")
            onesr = persist.tile([1, 128], bf16, tag="onesr")

            # collective bounce buffers (DRAM; SBUF collectives are broken)
            ib_k = dram.tile([C, QB], f32, tag="ib_k")
            ib_v = dram.tile([QB, C], f32, tag="ib_v")
            ob_k = dram.tile([NB, C, QB], f32, tag="ob_k")
            ob_v = dram.tile([NB, QB, C], f32, tag="ob_v")
            groups = [[0, 1, 2, 3], [4, 5, 6, 7]]

            nc.sync.dma_start(out=b_sb[:], in_=bqkv_d[:])
            nc.sync.dma_start(out=bproj[:], in_=bproj_d[:])
            nc.sync.dma_start(out=ident[:], in_=id_d[:])
            nc.gpsimd.dma_start(out=ones64[:], in_=ones64_d[:])
            nc.sync.dma_start(out=onesr[:], in_=onesr_d[:])
            nc.sync.dma_start(
                out=w2_sb[:],
                in_=w2_d[:].rearrange("(pc p) f -> p pc f", p=128))

            # ones columns of v_nat (softmax-denominator rows)
            dst = v_nat[:, 0, 0:1]
            nc.sync.dma_start(
                out=bass.AP(tensor=dst.tensor, offset=dst.offset + 64,
                            ap=[list(dst.ap[0]), [65, NKC * H]]),
                in_=ones16_d[:])

            def emit_body(rep):
                # ---- phase 1: local QKV for the core's own 512 tokens ----
                xt = xt_pool.tile([128, CC, QB], bf16, tag="xt",
                                  name=f"xt_{rep}")
                nc.sync.dma_start(
                    out=xt[:],
                    in_=xT_d[:].rearrange("(cc p) t -> p cc t", p=128))

                def proj_chunk(kind, oc, ps, half):
                    base = {"q": 0, "k": C, "v": 2 * C}[kind]
                    w_t = wst_pool.tile([128, CC, 128], bf16, tag="w",
                                        name=f"w_{kind}_{oc}_{rep}")
                    nc.sync.dma_start(
                        out=w_t[:],
                        in_=wqkv_d[:, base + oc * 128:base + (oc + 1) * 128]
                        .rearrange("(cc p) f -> p cc f", p=128))
                    sl = slice(half * 512, (half + 1) * 512)
                    for ci in range(CC):
                        nc.tensor.matmul(
                            ps[:, sl], w_t[:, ci, :], xt[:, ci, :],
                            start=(ci == 0), stop=(ci == CC - 1))

                # Q -> qT (f32r, stays local)
                for ocp in range(NOC // 2):
                    ps = psum_s.tile([128, 1024], f32, tag="s",
                                     name=f"psq_{ocp}_{rep}")
                    for half in range(2):
                        oc = ocp * 2 + half
                        proj_chunk("q", oc, ps, half)
                        nc.vector.tensor_scalar_add(
                            qT[:, oc, :], ps[:, half * 512:(half + 1) * 512],
                            b_sb[:, oc:oc + 1])

                # K -> rotating evac tiles -> DMA into the K bounce buffer
                for ocp in range(NOC // 2):
                    ps = psum_s.tile([128, 1024], f32, tag="s",
                                     name=f"psk_{ocp}_{rep}")
                    for half in range(2):
                        oc = ocp * 2 + half
                        proj_chunk("k", oc, ps, half)
                        kl = loc_pool.tile([128, QB], f32, tag="kl",
                                           name=f"kl_{oc}_{rep}")
                        nc.vector.tensor_scalar_add(
                            kl[:], ps[:, half * 512:(half + 1) * 512],
                            b_sb[:, 8 + oc:9 + oc])
                        nc.sync.dma_start(
                            out=ib_k[oc * 128:(oc + 1) * 128, :], in_=kl[:])

                # V -> vsb (channel-major), then local PE transpose into
                # natural layout per 128-token chunk -> V bounce buffer
                for ovp in range(NOC // 2):
                    ps = psum_s.tile([128, 1024], f32, tag="s",
                                     name=f"psv_{ovp}_{rep}")
                    for half in range(2):
                        ov = ovp * 2 + half
                        proj_chunk("v", ov, ps, half)
                        nc.vector.tensor_scalar_add(
                            vsb[:, ov, :],
                            ps[:, half * 512:(half + 1) * 512],
                            b_sb[:, 16 + ov:17 + ov])
                for tc_ in range(QB // 128):
                    pt = psum_s.tile([128, 1024], f32, tag="s",
                                     name=f"pt_{tc_}_{rep}")
                    for ov in range(NOC):
                        nc.tensor.transpose(
                            pt[:, ov * 128:(ov + 1) * 128],
                            vsb[:, ov, tc_ * 128:(tc_ + 1) * 128], ident[:])
                    vn = loc_pool.tile([128, C], f32, tag="vn",
                                       name=f"vn_{tc_}_{rep}")
                    nc.vector.tensor_copy(vn[:], pt[:])
                    nc.sync.dma_start(
                        out=ib_v[tc_ * 128:(tc_ + 1) * 128, :], in_=vn[:])

                # ---- gathers: full-batch K and V on every core ----
                nc.gpsimd.collective_compute(
                    "AllGather", mybir.AluOpType.bypass,
                    replica_groups=groups,
                    ins=[ib_k[:].opt()], outs=[ob_k[:].opt()])
                nc.gpsimd.collective_compute(
                    "AllGather", mybir.AluOpType.bypass,
                    replica_groups=groups,
                    ins=[ib_v[:].opt()], outs=[ob_v[:].opt()])

                # kT: [128, pair, 2048] (f32r cast; one DMA per member)
                for g in range(NB):
                    nc.gpsimd.dma_start(
                        out=kT[:, :, g * QB:(g + 1) * QB],
                        in_=ob_k[g].rearrange("(oc p) t -> p oc t", p=128))
                # v_nat: scatter gathered 128-token chunks into head slots
                for kc in range(NKC):
                    g, tc_ = kc // 4, kc % 4
                    vl = vld_pool.tile([128, C], f32, tag="vl",
                                       name=f"vl_{kc}_{rep}")
                    nc.sync.dma_start(
                        out=vl[:],
                        in_=ob_v[g, tc_ * 128:(tc_ + 1) * 128, :])
                    dstv = v_nat[:, kc, 0:1]
                    nc.vector.tensor_copy(
                        bass.AP(tensor=dstv.tensor, offset=dstv.offset,
                                ap=[list(dstv.ap[0]), [65, H], [1, 64]]),
                        bass.AP(tensor=vl.tensor, offset=vl.offset,
                                ap=[list(vl.ap[0]), [64, H], [1, 64]]))

                # ---- phase 2: attention per head pair ----
                def emit_attention(p):
                    po = [psum_o.tile([128, 512], f32, tag="po",
                                      name=f"po_{p}_{hh}_{rep}")
                          for hh in range(2)]
                    for kcg in range(NKC // 2):
                        exs = {}
                        for hh in range(2):
                            hsl = slice(hh * 64, (hh + 1) * 64)
                            ps = psum_s.tile([128, 1024], f32, tag="s",
                                             name=f"ps2_{p}_{kcg}_{hh}_{rep}")
                            for kc2 in range(2):
                                kc = kcg * 2 + kc2
                                nc.tensor.matmul(
                                    ps[:, kc2 * 512:(kc2 + 1) * 512],
                                    kT[hsl, p, kc * 128:(kc + 1) * 128],
                                    qT[hsl, p, :], start=True, stop=True)
                            ex = exp_pool.tile(
                                [128, 1024], bf16, tag="ex",
                                name=f"ex_{p}_{kcg}_{hh}_{rep}")
                            nc.scalar.activation(ex[:], ps[:], Exp,
                                                 scale=float(SCALE))
                            exs[hh] = ex
                        for kc2 in range(2):
                            kc = kcg * 2 + kc2
                            for hh in range(2):
                                h = 2 * p + hh
                                nc.tensor.matmul(
                                    po[hh][0:65, :],
                                    v_nat[:, kc, h * 65:h * 65 + 65],
                                    exs[hh][:, kc2 * 512:(kc2 + 1) * 512],
                                    start=(kc == 0), stop=(kc == NKC - 1))
                    for hh in range(2):
                        # softmax denominators: row 64 of po; broadcast
                        # across 64 partitions via a PE outer product, then
                        # DVE reciprocal + multiply into ohT
                        s_sb = small_pool.tile([1, 512], f32r, tag="r",
                                               name=f"s_sb_{p}_{hh}_{rep}")
                        nc.vector.tensor_copy(s_sb[:], po[hh][64:65, :])
                        pr = psum_r.tile([64, 512], f32, tag="pr",
                                         name=f"pr_{p}_{hh}_{rep}")
                        nc.tensor.matmul(pr[:], ones64[:], s_sb[:],
                                         start=True, stop=True)
                        rcp = small_pool.tile([64, 512], f32, tag="rb",
                                              name=f"rcp_{p}_{hh}_{rep}")
                        nc.vector.reciprocal(rcp[:], pr[:])
                        nc.vector.tensor_tensor(
                            ohT[hh * 64:(hh + 1) * 64, p, :],
                            po[hh][0:64, :], rcp[:], Mult)

                # ---- phase 3: output projection for the core's rows ----
                def emit_proj(tcg):
                    pp = psum_s.tile([128, 1024], f32, tag="s",
                                     name=f"pp_{tcg}_{rep}")
                    tsl = slice(tcg * 128, (tcg + 1) * 128)
                    for jh in range(2):
                        jsl = slice(jh * 512, (jh + 1) * 512)
                        for p in range(NP):
                            nc.tensor.matmul(pp[:, jsl], ohT[:, p, tsl],
                                             w2_sb[:, p, jsl],
                                             start=(p == 0), stop=False)
                        # bias via K=1 ones-row matmul
                        nc.tensor.matmul(pp[:, jsl], onesr[:],
                                         bproj[:, jsl],
                                         start=False, stop=True)
                    for jh in range(2):
                        ob = out_pool.tile([128, 512], bf16, tag="ob",
                                           name=f"ob_{tcg}_{jh}_{rep}")
                        nc.vector.tensor_copy(
                            ob[:], pp[:, jh * 512:(jh + 1) * 512])
                        nc.sync.dma_start(
                            out=out_d[tsl, jh * 512:(jh + 1) * 512],
                            in_=ob[:])

                for p in range(NP):
                    emit_attention(p)
                for tcg in range(QB // 128):
                    emit_proj(tcg)

            for rep in range(reps):
                emit_body(rep)

    nc.compile()
    return nc


def get_program(w_qkv=None, b_qkv=None, w_proj=None, b_proj=None, reps=1):
    import hashlib
    ws = [np.ascontiguousarray(np.asarray(a, np.float32))
          for a in (w_qkv, b_qkv, w_proj, b_proj)]
    key = (hashlib.sha1(b"".join(a.tobytes() for a in ws)).hexdigest(), reps)
    if key not in _CACHE:
        _CACHE[key] = _build_program(*ws, reps=reps)
    return _CACHE[key]


def build_null_program():
    """Tiny kernel for calibrating per-dispatch overhead in test harnesses."""
    import concourse.mybir as mybir
    import concourse.tile as tile
    from concourse import bacc

    f32 = mybir.dt.float32
    nc = bacc.Bacc("TRN2", target_bir_lowering=False, debug=False,
                   num_devices=NCORES)
    x_in = nc.dram_tensor("x", [128, 128], f32, kind="ExternalInput")
    y_out = nc.dram_tensor("y", [128, 128], f32, kind="ExternalOutput")
    with tile.TileContext(nc) as tc:
        with tc.tile_pool(name="p", bufs=1) as pool:
            t = pool.tile([128, 128], f32)
            nc.sync.dma_start(out=t[:], in_=x_in[:])
            nc.sync.dma_start(out=y_out[:], in_=t[:])
    nc.compile()
    x = np.zeros((128, 128), dtype=np.float32)
    return nc, [{"x": x} for _ in range(NCORES)]


def make_in_maps(x, *unused):
    """Host-side sharding: per-core input dicts (own 512 tokens each)."""
    import ml_dtypes
    x = np.asarray(x, np.float32)
    in_maps = []
    for core in range(NCORES):
        b, g = core // NB, core % NB
        xq = x[b, g * QB:(g + 1) * QB]
        in_maps.append({"xqT": np.ascontiguousarray(xq.T).astype(
            ml_dtypes.bfloat16)})
    return in_maps


def combine_results(results, b_proj=None):
    """Host-side unshard: concatenate the disjoint 512-row slices."""
    out = np.empty((B, N, C), dtype=np.float32)
    for core in range(NCORES):
        b, g = core // NB, core % NB
        out[b, g * QB:(g + 1) * QB, :] = np.asarray(
            results[core]["out_q"], dtype=np.float32)
    return out


def kernel(x, w_qkv, b_qkv, w_proj, b_proj):
    from concourse.bass_utils import run_bass_kernel_spmd

    nc = get_program(w_qkv, b_qkv, w_proj, b_proj)
    in_maps = make_in_maps(x)
    res = run_bass_kernel_spmd(nc, in_maps, list(range(NCORES)))
    return combine_results(res.results)


# revision 27
# speedup vs baseline: 1.0431x; 1.0431x over previous
"""Multi-head self-attention Trainium2 kernel (8-core token-parallel with
collective K/V sharing).

Problem: B=2, N=2048, C=1024, H=16 heads, HD=64.

Sharding: data-parallel over tokens -- core c owns 512 tokens (batch c//4,
block c%4). Each core:

  1. Computes Q, K, V for its OWN 512 tokens only (all 16 heads). Weights
     are Const tensors baked into the NEFF (weight-stationary); the only
     per-call input is the core's 1MB bf16 x^T slice.
  2. AllGathers K (channel-major) and V (natural token-major layout,
     transposed locally on the PE before the gather) across the 4 cores of
     its batch, so every core sees the full 2048-token K/V.
  3. Runs full softmax attention for its 512 queries (no max subtraction:
     |s*scale| < ~10 for this distribution) and the output projection for
     its 512 rows. Outputs are disjoint -- the host just concatenates.

Matmuls run bf16 for QKV, f32r for scores, bf16 for attn@v and the
projection. Per-head softmax denominators come for free as row 64 of the
attn@v accumulator via a constant-ones column appended to v.

`reps` repeats the whole body inside one NEFF so a test harness can
measure device execution time as a slope, cancelling dispatch overhead.
"""

import numpy as np

B, N, C = 2, 2048, 1024
H = 16
HD = C // H  # 64
SCALE = HD ** -0.5
NCORES = 8
QB = 512                      # tokens owned per core
NB = N // QB                  # 4 cores per batch

_CACHE = {}


def _build_program(w_qkv, b_qkv, w_proj, b_proj, reps=1):
    import ml_dtypes
    import concourse.bass as bass
    import concourse.mybir as mybir
    import concourse.tile as tile
    from concourse import bacc

    f32 = mybir.dt.float32
    f32r = mybir.dt.float32r
    bf16 = mybir.dt.bfloat16
    Exp = mybir.ActivationFunctionType.Exp
    Mult = mybir.AluOpType.mult

    nc = bacc.Bacc("TRN2", target_bir_lowering=False, debug=False,
                   num_devices=NCORES)

    # per-call input: this core's own 512 tokens, transposed (bf16)
    xT_d = nc.dram_tensor("xqT", [C, QB], bf16, kind="ExternalInput")
    out_d = nc.dram_tensor("out_q", [QB, C], bf16, kind="ExternalOutput")

    # baked weights (identical on every core)
    wqkv_d = nc.inline_tensor(
        np.ascontiguousarray(w_qkv).astype(ml_dtypes.bfloat16),
        name="wqkv_c")
    w2_d = nc.inline_tensor(
        np.ascontiguousarray(w_proj).astype(ml_dtypes.bfloat16), name="w2_c")
    bqkv_d = nc.inline_tensor(np.ascontiguousarray(
        b_qkv.reshape(24, 128).T, np.float32), name="bqkv_c")
    bproj_d = nc.inline_tensor(
        b_proj.reshape(1, C).astype(ml_dtypes.bfloat16), name="bproj_c")
    id_d = nc.inline_tensor(np.eye(128, dtype=np.float32), name="ident_c")
    ones64_d = nc.inline_tensor(np.ones((1, 64), np.float32), name="o64_c")
    onesr_d = nc.inline_tensor(np.ones((1, 128), ml_dtypes.bfloat16),
                               name="or_c")
    ones16_d = nc.inline_tensor(np.ones((128, 256), ml_dtypes.bfloat16),
                                name="o16_c")

    CC = C // 128      # 8 contraction chunks
    NOC = C // 128     # 8 output chunks per projection (q, k or v)
    NKC = N // 128     # 16 key chunks (full batch)
    NP = H // 2        # 8 head pairs (one 128-row chunk each)

    with tile.TileContext(nc) as tc:
        with tc.tile_pool(name="persist", bufs=1) as persist, \
             tc.tile_pool(name="xt", bufs=1) as xt_pool, \
             tc.tile_pool(name="wst", bufs=2) as wst_pool, \
             tc.tile_pool(name="loc", bufs=2) as loc_pool, \
             tc.tile_pool(name="exp", bufs=3) as exp_pool, \
             tc.tile_pool(name="vld", bufs=2) as vld_pool, \
             tc.tile_pool(name="small", bufs=2) as small_pool, \
             tc.tile_pool(name="ob", bufs=2) as out_pool, \
             tc.tile_pool(name="dram", bufs=1, space="DRAM") as dram, \
             tc.tile_pool(name="ps", bufs=2, space="PSUM") as psum_s, \
             tc.tile_pool(name="po", bufs=2, space="PSUM") as psum_o, \
             tc.tile_pool(name="pr", bufs=2, space="PSUM") as psum_r:

            kT = persist.tile([128, NP, N], bf16, tag="kT")
            qT = persist.tile([128, NP, QB], bf16, tag="qT")
            # natural-layout v per key chunk: per head [v(64) | 1.0]
            v_nat = persist.tile([128, NKC, H * 65], bf16, tag="v_nat")
            w2_sb = persist.tile([128, CC, C], bf16, tag="w2_sb")
            ohT = persist.tile([128, NP, QB], bf16, tag="ohT")
            vsb = persist.tile([128, NOC, QB], f32, tag="vsb")
            b_sb = persist.tile([128, 24], f32, tag="b_sb")
            bproj = persist.tile([1, C], bf16, tag="bproj")
            ident = persist.tile([128, 128], f32, tag="ident")
            ones64 = persist.tile([1, 64], f32r, tag="ones64")
            onesr = persist.tile([1, 128], bf16, tag="onesr")

            # merged K+V collective bounce buffers (DRAM, bf16; SBUF
            # collectives are broken). Slot 0 holds K flat as [C, QB];
            # slot 1 holds V natural [QB, C].
            ib_kv = dram.tile([2, QB, C], bf16, tag="ib_kv")
            ob_kv = dram.tile([NB, 2, QB, C], bf16, tag="ob_kv")
            groups = [[0, 1, 2, 3], [4, 5, 6, 7]]

            nc.sync.dma_start(out=b_sb[:], in_=bqkv_d[:])
            nc.sync.dma_start(out=bproj[:], in_=bproj_d[:])
            nc.sync.dma_start(out=ident[:], in_=id_d[:])
            nc.gpsimd.dma_start(out=ones64[:], in_=ones64_d[:])
            nc.sync.dma_start(out=onesr[:], in_=onesr_d[:])
            nc.sync.dma_start(
                out=w2_sb[:],
                in_=w2_d[:].rearrange("(pc p) f -> p pc f", p=128))

            # ones columns of v_nat (softmax-denominator rows)
            dst = v_nat[:, 0, 0:1]
            nc.sync.dma_start(
                out=bass.AP(tensor=dst.tensor, offset=dst.offset + 64,
                            ap=[list(dst.ap[0]), [65, NKC * H]]),
                in_=ones16_d[:])

            def emit_body(rep):
                # ---- phase 1: local QKV for the core's own 512 tokens ----
                xt = xt_pool.tile([128, CC, QB], bf16, tag="xt",
                                  name=f"xt_{rep}")
                nc.sync.dma_start(
                    out=xt[:],
                    in_=xT_d[:].rearrange("(cc p) t -> p cc t", p=128))

                def proj_chunk(kind, oc, ps, half):
                    base = {"q": 0, "k": C, "v": 2 * C}[kind]
                    w_t = wst_pool.tile([128, CC, 128], bf16, tag="w",
                                        name=f"w_{kind}_{oc}_{rep}")
                    nc.sync.dma_start(
                        out=w_t[:],
                        in_=wqkv_d[:, base + oc * 128:base + (oc + 1) * 128]
                        .rearrange("(cc p) f -> p cc f", p=128))
                    sl = slice(half * 512, (half + 1) * 512)
                    for ci in range(CC):
                        nc.tensor.matmul(
                            ps[:, sl], w_t[:, ci, :], xt[:, ci, :],
                            start=(ci == 0), stop=(ci == CC - 1))

                # Q -> qT (f32r, stays local)
                for ocp in range(NOC // 2):
                    ps = psum_s.tile([128, 1024], f32, tag="s",
                                     name=f"psq_{ocp}_{rep}")
                    for half in range(2):
                        oc = ocp * 2 + half
                        proj_chunk("q", oc, ps, half)
                        nc.vector.tensor_scalar_add(
                            qT[:, oc, :], ps[:, half * 512:(half + 1) * 512],
                            b_sb[:, oc:oc + 1])

                # K -> rotating evac tiles -> slot 0 of the bounce buffer
                # (flat [C, QB] layout via a raw AP)
                for ocp in range(NOC // 2):
                    ps = psum_s.tile([128, 1024], f32, tag="s",
                                     name=f"psk_{ocp}_{rep}")
                    for half in range(2):
                        oc = ocp * 2 + half
                        proj_chunk("k", oc, ps, half)
                        kl = loc_pool.tile([128, QB], bf16, tag="kl",
                                           name=f"kl_{oc}_{rep}")
                        nc.vector.tensor_scalar_add(
                            kl[:], ps[:, half * 512:(half + 1) * 512],
                            b_sb[:, 8 + oc:9 + oc])
                        kbase = ib_kv[0, 0, 0:1]
                        nc.sync.dma_start(
                            out=bass.AP(tensor=kbase.tensor,
                                        offset=kbase.offset + oc * 128 * QB,
                                        ap=[[QB, 128], [1, QB]]),
                            in_=kl[:])

                # V -> vsb (channel-major), then local PE transpose into
                # natural layout per 128-token chunk -> V bounce buffer
                for ovp in range(NOC // 2):
                    ps = psum_s.tile([128, 1024], f32, tag="s",
                                     name=f"psv_{ovp}_{rep}")
                    for half in range(2):
                        ov = ovp * 2 + half
                        proj_chunk("v", ov, ps, half)
                        nc.vector.tensor_scalar_add(
                            vsb[:, ov, :],
                            ps[:, half * 512:(half + 1) * 512],
                            b_sb[:, 16 + ov:17 + ov])
                for tc_ in range(QB // 128):
                    pt = psum_s.tile([128, 1024], f32, tag="s",
                                     name=f"pt_{tc_}_{rep}")
                    for ov in range(NOC):
                        nc.tensor.transpose(
                            pt[:, ov * 128:(ov + 1) * 128],
                            vsb[:, ov, tc_ * 128:(tc_ + 1) * 128], ident[:])
                    vn = loc_pool.tile([128, C], bf16, tag="vn",
                                       name=f"vn_{tc_}_{rep}")
                    nc.vector.tensor_copy(vn[:], pt[:])
                    nc.sync.dma_start(
                        out=ib_kv[1, tc_ * 128:(tc_ + 1) * 128, :],
                        in_=vn[:])

                # ---- single gather: full-batch K+V on every core ----
                nc.gpsimd.collective_compute(
                    "AllGather", mybir.AluOpType.bypass,
                    replica_groups=groups,
                    ins=[ib_kv[:].opt()], outs=[ob_kv[:].opt()])

                # kT: [128, pair, 2048] (one DMA per member; raw AP reads
                # the flat [C, QB] K slab of member g)
                for g in range(NB):
                    gbase = ob_kv[g, 0, 0, 0:1]
                    nc.sync.dma_start(
                        out=kT[:, :, g * QB:(g + 1) * QB],
                        in_=bass.AP(tensor=gbase.tensor, offset=gbase.offset,
                                    ap=[[QB, 128], [128 * QB, NOC],
                                        [1, QB]]))
                # v_nat: scatter gathered 128-token chunks into head slots
                for kc in range(NKC):
                    g, tc_ = kc // 4, kc % 4
                    vl = vld_pool.tile([128, C], bf16, tag="vl",
                                       name=f"vl_{kc}_{rep}")
                    nc.sync.dma_start(
                        out=vl[:],
                        in_=ob_kv[g, 1, tc_ * 128:(tc_ + 1) * 128, :])
                    dstv = v_nat[:, kc, 0:1]
                    nc.vector.tensor_copy(
                        bass.AP(tensor=dstv.tensor, offset=dstv.offset,
                                ap=[list(dstv.ap[0]), [65, H], [1, 64]]),
                        bass.AP(tensor=vl.tensor, offset=vl.offset,
                                ap=[list(vl.ap[0]), [64, H], [1, 64]]))

                # ---- phase 2: attention per head pair ----
                def emit_attention(p):
                    po = [psum_o.tile([128, 512], f32, tag="po",
                                      name=f"po_{p}_{hh}_{rep}")
                          for hh in range(2)]
                    for kcg in range(NKC // 2):
                        exs = {}
                        for hh in range(2):
                            hsl = slice(hh * 64, (hh + 1) * 64)
                            ps = psum_s.tile([128, 1024], f32, tag="s",
                                             name=f"ps2_{p}_{kcg}_{hh}_{rep}")
                            for kc2 in range(2):
                                kc = kcg * 2 + kc2
                                nc.tensor.matmul(
                                    ps[:, kc2 * 512:(kc2 + 1) * 512],
                                    kT[hsl, p, kc * 128:(kc + 1) * 128],
                                    qT[hsl, p, :], start=True, stop=True)
                            ex = exp_pool.tile(
                                [128, 1024], bf16, tag="ex",
                                name=f"ex_{p}_{kcg}_{hh}_{rep}")
                            nc.scalar.activation(ex[:], ps[:], Exp,
                                                 scale=float(SCALE))
                            exs[hh] = ex
                        for kc2 in range(2):
                            kc = kcg * 2 + kc2
                            for hh in range(2):
                                h = 2 * p + hh
                                nc.tensor.matmul(
                                    po[hh][0:65, :],
                                    v_nat[:, kc, h * 65:h * 65 + 65],
                                    exs[hh][:, kc2 * 512:(kc2 + 1) * 512],
                                    start=(kc == 0), stop=(kc == NKC - 1))
                    for hh in range(2):
                        # softmax denominators: row 64 of po; broadcast
                        # across 64 partitions via a PE outer product, then
                        # DVE reciprocal + multiply into ohT
                        s_sb = small_pool.tile([1, 512], f32r, tag="r",
                                               name=f"s_sb_{p}_{hh}_{rep}")
                        nc.vector.tensor_copy(s_sb[:], po[hh][64:65, :])
                        pr = psum_r.tile([64, 512], f32, tag="pr",
                                         name=f"pr_{p}_{hh}_{rep}")
                        nc.tensor.matmul(pr[:], ones64[:], s_sb[:],
                                         start=True, stop=True)
                        rcp = small_pool.tile([64, 512], f32, tag="rb",
                                              name=f"rcp_{p}_{hh}_{rep}")
                        nc.vector.reciprocal(rcp[:], pr[:])
                        nc.vector.tensor_tensor(
                            ohT[hh * 64:(hh + 1) * 64, p, :],
                            po[hh][0:64, :], rcp[:], Mult)

                # ---- phase 3: output projection for the core's rows ----
                def emit_proj(tcg):
                    pp = psum_s.tile([128, 1024], f32, tag="s",
                                     name=f"pp_{tcg}_{rep}")
                    tsl = slice(tcg * 128, (tcg + 1) * 128)
                    for jh in range(2):
                        jsl = slice(jh * 512, (jh + 1) * 512)
                        for p in range(NP):
                            nc.tensor.matmul(pp[:, jsl], ohT[:, p, tsl],
                                             w2_sb[:, p, jsl],
                                             start=(p == 0), stop=False)
                        # bias via K=1 ones-row matmul
                        nc.tensor.matmul(pp[:, jsl], onesr[:],
                                         bproj[:, jsl],
                                         start=False, stop=True)
                    for jh in range(2):
                        ob = out_pool.tile([128, 512], bf16, tag="ob",
                                           name=f"ob_{tcg}_{jh}_{rep}")
                        nc.vector.tensor_copy(
                            ob[:], pp[:, jh * 512:(jh + 1) * 512])
                        nc.sync.dma_start(
                            out=out_d[tsl, jh * 512:(jh + 1) * 512],
                            in_=ob[:])

                for p in range(NP):
                    emit_attention(p)
                for tcg in range(QB // 128):
                    emit_proj(tcg)

            for rep in range(reps):
                emit_body(rep)

    nc.compile()
    return nc


def get_program(w_qkv=None, b_qkv=None, w_proj=None, b_proj=None, reps=1):
    import hashlib
    ws = [np.ascontiguousarray(np.asarray(a, np.float32))
          for a in (w_qkv, b_qkv, w_proj, b_proj)]
    key = (hashlib.sha1(b"".join(a.tobytes() for a in ws)).hexdigest(), reps)
    if key not in _CACHE:
        _CACHE[key] = _build_program(*ws, reps=reps)
    return _CACHE[key]


def build_null_program():
    """Tiny kernel for calibrating per-dispatch overhead in test harnesses."""
    import concourse.mybir as mybir
    import concourse.tile as tile
    from concourse import bacc

    f32 = mybir.dt.float32
    nc = bacc.Bacc("TRN2", target_bir_lowering=False, debug=False,
                   num_devices=NCORES)
    x_in = nc.dram_tensor("x", [128, 128], f32, kind="ExternalInput")
    y_out = nc.dram_tensor("y", [128, 128], f32, kind="ExternalOutput")
    with tile.TileContext(nc) as tc:
        with tc.tile_pool(name="p", bufs=1) as pool:
            t = pool.tile([128, 128], f32)
            nc.sync.dma_start(out=t[:], in_=x_in[:])
            nc.sync.dma_start(out=y_out[:], in_=t[:])
    nc.compile()
    x = np.zeros((128, 128), dtype=np.float32)
    return nc, [{"x": x} for _ in range(NCORES)]


def make_in_maps(x, *unused):
    """Host-side sharding: per-core input dicts (own 512 tokens each)."""
    import ml_dtypes
    x = np.asarray(x, np.float32)
    in_maps = []
    for core in range(NCORES):
        b, g = core // NB, core % NB
        xq = x[b, g * QB:(g + 1) * QB]
        in_maps.append({"xqT": np.ascontiguousarray(xq.T).astype(
            ml_dtypes.bfloat16)})
    return in_maps


def combine_results(results, b_proj=None):
    """Host-side unshard: concatenate the disjoint 512-row slices."""
    out = np.empty((B, N, C), dtype=np.float32)
    for core in range(NCORES):
        b, g = core // NB, core % NB
        out[b, g * QB:(g + 1) * QB, :] = np.asarray(
            results[core]["out_q"], dtype=np.float32)
    return out


def kernel(x, w_qkv, b_qkv, w_proj, b_proj):
    from concourse.bass_utils import run_bass_kernel_spmd

    nc = get_program(w_qkv, b_qkv, w_proj, b_proj)
    in_maps = make_in_maps(x)
    res = run_bass_kernel_spmd(nc, in_maps, list(range(NCORES)))
    return combine_results(res.results)
